# revision 16
# baseline (speedup 1.0000x reference)
"""Trainium2 Bass kernel for CSPNet-style GNN message passing (128 graphs x 24 atoms).

Strategy (graph-parallel over 8 cores, 16 graphs/core):
  - Edges are fully-connected per graph (24x24 incl. self loops) -> deg=24, and the
    edge MLP's first matmul decomposes over e_in = [hn[src], hn[dst], lat_e, dis]:
        z1 = A[src] + B[dst] + C[g] + dis @ W1d + b1
    with A = hn@W1a, B = hn@W1b computed at NODE level (24x fewer flops), and the
    src/dst gathers realized as zero-stride broadcast access patterns (no data mvmt).
  - dis (sin/cos positional features) computed once into DRAM as bf16; range-reduced
    via (f*t mod 1) so ACT Sin sees [-pi, pi].
  - All big matmuls run in bf16 (full PE rate + fast weight load); psum fp32.
  - Feature-major ("fm") layout [feat_on_partitions, tokens] for all matmul chains;
    node-major only for LayerNorm; PE-transpose bridges the two once per layer.
  - scatter_mean folds into a free-dim strided reduce (blocks of 24) + 1/24 folded
    into n_w1 rows on the host.
  - Edge loop software-pipelined: z1 of tile t is issued before W2 of tile t-1 so
    the PE never waits on the DVE-add + silu chain of the current tile.
  - Per-layer weights double-buffered and prefetched from inside the previous
    layer's edge loop.
"""

import os
import sys

import numpy as np

if "/opt/trn_rl_repo" not in sys.path:
    sys.path.insert(0, "/opt/trn_rl_repo")

import concourse.bass as bass
import concourse.tile as tile
from concourse import bacc, mybir

f32 = mybir.dt.float32
bf16 = mybir.dt.bfloat16
i32 = mybir.dt.int32
AF = mybir.ActivationFunctionType
ALU = mybir.AluOpType
AX = mybir.AxisListType

N_GRAPHS = 128
ATOMS = 24
N = N_GRAPHS * ATOMS
H = 512
L = 6
NFREQ = 128
MAXEL = 100
NCORES = 8
GPC = N_GRAPHS // NCORES          # 16 graphs per core
NPC = GPC * ATOMS                 # 384 nodes per core
EPC = GPC * ATOMS * ATOMS         # 9216 edges per core
HALF = ATOMS * ATOMS // 2         # 288 edges per tile (12 src blocks)
NBLK = 12                         # src blocks per half-graph tile
NTILE = 2 * GPC                   # 32 edge tiles per layer

# column offsets inside the edge weight tile wlE [128, 9216] bf16
OFF_AB = 0              # 8 x 512   (W1a k=0..3, W1b k=0..3)
OFF_D = 8 * 512         # 6 x 512   (W1d)
OFF_W2 = OFF_D + 6 * 512   # 4 x 512
WLE_COLS = OFF_W2 + 4 * 512   # 9216
# node weight tile wlN [128, 6144] bf16
OFF_N1 = 0              # 8 x 512
OFF_N2 = 8 * 512        # 4 x 512
WLN_COLS = OFF_N2 + 4 * 512   # 6144


def build(nc: bass.Bass, sim_silu: bool = False):
    """Trace the per-core program. Same program for all 8 cores (SPMD)."""
    din = {}
    for name, shape, dt in [
        ("xT", [3, NPC], f32), ("oneh", [MAXEL, NPC], f32), ("lat", [GPC, 9], f32),
        ("eye", [128, 128], f32), ("eyeb", [128, 128], bf16), ("fcol", [128, 1], f32),
        ("embt", [MAXEL, H], f32),
        ("ew1", [L, 2 * H + 9 + 6 * NFREQ, H], bf16), ("ew2", [L, H, H], bf16),
        ("w1cb", [L, 10, H], bf16),
        ("nw1", [L, 2 * H, H], bf16), ("nw2", [L, H, H], bf16),
        ("lnw", [L, H], f32), ("lnb", [L, H], f32),
        ("eb2", [L, H], f32), ("nb1", [L, H], f32), ("nb2", [L, H], bf16),
        ("flnw", [H], f32), ("flnb", [H], f32),
        ("outw", [H, H], bf16), ("outb", [H], bf16),
        ("ones1", [1, 128], bf16),
    ]:
        din[name] = nc.dram_tensor(name, shape, dt, kind="ExternalInput")
    hout = nc.dram_tensor("hout", [NPC, H], f32, kind="ExternalOutput")

    def silu1(pool, out_ap, in_ap, bias=0.0):
        """out = Silu(in + bias). bias: [128,1] AP or float."""
        if not sim_silu:
            nc.scalar.activation(out_ap, in_ap, AF.Silu, bias=bias, scale=1.0)
        else:  # CoreSim lacks Silu: Identity(+bias) -> Sigmoid -> mul
            t1 = pool.tile(list(in_ap.shape), f32, tag="sims1", bufs=2)
            nc.scalar.activation(t1[:], in_ap, AF.Identity, bias=bias, scale=1.0)
            t2 = pool.tile(list(in_ap.shape), f32, tag="sims2", bufs=2)
            nc.scalar.activation(t2[:], t1[:], AF.Sigmoid)
            nc.vector.tensor_tensor(out=out_ap, in0=t1[:], in1=t2[:], op=ALU.mult)

    def silu(pool, out_ap, in_ap, bias=0.0, bias2=None):
        """Silu over [128, 2, E] m-pair views when bias2 given, else single."""
        if bias2 is None:
            silu1(pool, out_ap, in_ap, bias)
        else:
            silu1(pool, out_ap[:, 0], in_ap[:, 0], bias)
            silu1(pool, out_ap[:, 1], in_ap[:, 1], bias2)

    with tile.TileContext(nc) as tc:
        with (
            tc.tile_pool(name="const", bufs=1) as cpool,
            tc.tile_pool(name="wl", bufs=1) as wpool,
            tc.tile_pool(name="node", bufs=1) as npool,
            tc.tile_pool(name="ln", bufs=1) as lnpool,
            tc.tile_pool(name="edge", bufs=1) as epool,
            tc.tile_pool(name="small", bufs=1) as spool,
            tc.tile_pool(name="dram", bufs=1, space="DRAM") as dpool,
            tc.tile_pool(name="psZ", bufs=1, space="PSUM") as ppZ,
            tc.tile_pool(name="psS", bufs=1, space="PSUM") as ppS,
        ):
            # ---- constants / setup ----
            eye = cpool.tile([128, 128], f32)
            nc.sync.dma_start(eye[:], din["eye"][:])
            eyeb = cpool.tile([128, 128], bf16)
            nc.sync.dma_start(eyeb[:], din["eyeb"][:])
            fcol = cpool.tile([128, 1], f32)
            nc.sync.dma_start(fcol[:], din["fcol"][:])
            ones1 = cpool.tile([1, 128], bf16)
            nc.sync.dma_start(ones1[:], din["ones1"][:])
            c_eps = cpool.tile([128, 1], f32)
            nc.vector.memset(c_eps[:], 1e-5)
            c_hpi = cpool.tile([128, 1], f32)
            nc.vector.memset(c_hpi[:], float(np.pi / 2))
            c_pi = cpool.tile([128, 1], f32)
            nc.vector.memset(c_pi[:], float(np.pi))
            xt = cpool.tile([3, NPC], f32)
            nc.sync.dma_start(xt[:], din["xT"][:])
            oneh = cpool.tile([MAXEL, NPC], f32)
            nc.sync.dma_start(oneh[:], din["oneh"][:])
            embt = cpool.tile([MAXEL, H], f32)
            nc.sync.dma_start(embt[:], din["embt"][:])
            latt = cpool.tile([GPC, 9], f32)
            nc.sync.dma_start(latt[:], din["lat"][:])
            outw = cpool.tile([128, 4 * H], bf16)
            for k in range(4):
                nc.sync.dma_start(outw[:, k * H:(k + 1) * H],
                                  din["outw"][k * 128:(k + 1) * 128, :])
            outbr = cpool.tile([1, H], bf16)
            nc.sync.dma_start(outbr[:], din["outb"][:].rearrange("(o h) -> o h", o=1))
            flnt = cpool.tile([128, 8], f32)
            nc.sync.dma_start(flnt[:, 0:4], din["flnw"][:].rearrange("(m p) -> p m", p=128))
            nc.sync.dma_start(flnt[:, 4:8], din["flnb"][:].rearrange("(m p) -> p m", p=128))

            # fd[c, e=(g,i,j)] = x[dst=j, c] - x[src=i, c] + 1  (stored in DRAM)
            fd_d = dpool.tile([3, EPC], f32)
            for g in range(GPC):
                sl = xt[:, g * ATOMS:(g + 1) * ATOMS]
                fds = spool.tile([3, ATOMS * ATOMS], f32, tag="fds", bufs=1)
                nc.vector.tensor_tensor(
                    out=fds[:].rearrange("p (i j) -> p i j", j=ATOMS),
                    in0=sl.rearrange("p (o j) -> p o j", o=1).broadcast_to([3, ATOMS, ATOMS]),
                    in1=sl.broadcast_to([3, ATOMS, ATOMS]),
                    op=ALU.subtract)
                nc.vector.tensor_scalar(out=fds[:], in0=fds[:], scalar1=1.0,
                                        scalar2=None, op0=ALU.add)
                nc.sync.dma_start(fd_d[:, g * ATOMS * ATOMS:(g + 1) * ATOMS * ATOMS], fds[:])

            # dis features (layer-independent), computed per tile, stored bf16:
            # r = f*t - rne(f*t) in [-0.5,0.5];  sin chunk = Sin(2*pi*r);
            # cos chunk = Sin(pi/2 - 2*pi*|r|) = cos(2*pi*r).  HW Sin is only
            # accurate for |arg| <= pi (measured), so the abs trick is required.
            # Work is spread over ACT (mult via scale, abs, sins), DVE (int
            # round-trip) and GPSIMD (subtract) so layer 0 can interleave this
            # with its edge tiles without starving any one engine.
            dis_d = dpool.tile([128, NTILE * 6 * HALF], bf16)

            def dis_compute(t):
                """Emit setup for dis tile t; returns the SBUF bf16 tile."""
                e0 = t * HALF
                emb3 = epool.tile([128, 3 * HALF], f32, tag="emb3", bufs=2)
                nc.sync.dma_start(
                    emb3[:].rearrange("p (c e) -> p c e", e=HALF),
                    fd_d[:, e0:e0 + HALF].partition_broadcast(128))
                embm = epool.tile([128, 3 * HALF], f32, tag="embm", bufs=2)
                nc.scalar.activation(embm[:], emb3[:], AF.Identity, scale=fcol[:])
                vi = epool.tile([128, 3 * HALF], i32, tag="vi", bufs=2)
                nc.vector.tensor_copy(vi[:], embm[:])
                nc.gpsimd.tensor_tensor(out=embm[:], in0=embm[:], in1=vi[:],
                                        op=ALU.subtract)
                rabs = epool.tile([128, 3 * HALF], f32, tag="rabs", bufs=2)
                nc.scalar.activation(rabs[:], embm[:], AF.Abs)
                diss = epool.tile([128, 6 * HALF], bf16, tag="diss", bufs=3)
                for c in range(3):
                    nc.scalar.activation(diss[:, c * HALF:(c + 1) * HALF],
                                         embm[:, c * HALF:(c + 1) * HALF],
                                         AF.Sin, scale=2 * np.pi)
                    nc.scalar.activation(diss[:, (3 + c) * HALF:(4 + c) * HALF],
                                         rabs[:, c * HALF:(c + 1) * HALF],
                                         AF.Sin, scale=-2 * np.pi, bias=c_hpi[:])
                nc.sync.dma_start(dis_d[:, t * 6 * HALF:(t + 1) * 6 * HALF], diss[:])
                return diss

            # lattice gram: lat_ip[g, i*3+k] = sum_j L[g,i,j] L[g,k,j]
            # -> latfm10 [10, GPC] bf16 with a trailing ones row (for the +e_b1 fold)
            lt27 = spool.tile([GPC, 27], f32, tag="lt27")
            nc.vector.tensor_tensor(
                out=lt27[:].rearrange("p (i k j) -> p i k j", k=3, j=3),
                in0=latt[:].rearrange("p (i o j) -> p i o j", o=1, j=3)
                    .broadcast_to([GPC, 3, 3, 3]),
                in1=latt[:].rearrange("p (o k j) -> p o k j", o=1, j=3)
                    .broadcast_to([GPC, 3, 3, 3]),
                op=ALU.mult)
            latnm = spool.tile([GPC, 9], f32, tag="latnm")
            nc.vector.tensor_reduce(out=latnm[:], in_=lt27[:].rearrange(
                "p (ik j) -> p ik j", j=3), axis=AX.X, op=ALU.add)
            plat = ppS.tile([9, GPC], f32, tag="ps", bufs=2)
            nc.tensor.transpose(plat[:], latnm[:], eye[:GPC, :GPC])
            latfm10 = cpool.tile([10, GPC], bf16)
            nc.vector.memset(latfm10[:], 1.0)
            nc.scalar.copy(latfm10[0:9, :], plat[:])

            # h0 = emb_table[atom_types]  (node-major, via one-hot matmul, exact fp32)
            hbuf = npool.tile([128, 3 * H], f32)
            for p in range(3):
                ph = ppS.tile([128, H], f32, tag="ps", bufs=2)
                nc.tensor.matmul(ph[:], oneh[:, p * 128:(p + 1) * 128], embt[:],
                                 start=True, stop=True)
                nc.scalar.copy(hbuf[:, p * H:(p + 1) * H], ph[:])

            def load_weights(l):
                """DMA layer l weights into double-buffered SBUF tiles."""
                wlE = wpool.tile([128, WLE_COLS], bf16, tag="wlE", bufs=2)
                wlN = wpool.tile([128, WLN_COLS], bf16, tag="wlN", bufs=2)

                def wload(dst, off, dram_rows, nk):
                    nc.sync.dma_start(
                        dst[:, off:off + nk * 512].rearrange("p (k m) -> p k m", m=512),
                        dram_rows.rearrange("(k p) m -> p k m", p=128))
                wload(wlE, OFF_AB, din["ew1"][l, 0:1024, :], 8)        # W1a+W1b
                wload(wlE, OFF_D, din["ew1"][l, 1033:1801, :], 6)      # W1d
                wload(wlE, OFF_W2, din["ew2"][l], 4)                   # e_w2
                wload(wlN, OFF_N1, din["nw1"][l], 8)                   # n_w1 (pre-scaled)
                wload(wlN, OFF_N2, din["nw2"][l], 4)                   # n_w2
                w1cbt = wpool.tile([10, H], bf16, tag="w1cb", bufs=2)
                nc.sync.dma_start(w1cbt[:], din["w1cb"][l])
                bl = wpool.tile([128, 16], f32, tag="bl", bufs=2)
                for j, nm in enumerate(["lnw", "lnb", "eb2", "nb1"]):
                    nc.sync.dma_start(bl[:, j * 4:(j + 1) * 4],
                                      din[nm][l].rearrange("(m p) -> p m", p=128))
                nb2r = wpool.tile([1, H], bf16, tag="nb2r", bufs=2)
                nc.sync.dma_start(nb2r[:], din["nb2"][l].rearrange("(o h) -> o h", o=1))
                return dict(wlE=wlE, wlN=wlN, w1cbt=w1cbt, bl=bl, nb2r=nb2r)

            wt = load_weights(0)

            def ln_stats(p):
                """LayerNorm stats for node chunk p of hbuf -> xn (bf16)."""
                hs = hbuf[:, p * H:(p + 1) * H]
                musum = spool.tile([128, 1], f32, tag="musum", bufs=4)
                nc.vector.tensor_reduce(out=musum[:], in_=hs, axis=AX.X, op=ALU.add)
                mu = spool.tile([128, 1], f32, tag="mu", bufs=4)
                nc.scalar.activation(mu[:], musum[:], AF.Identity, scale=1.0 / H)
                sq = lnpool.tile([128, H], f32, tag="sq", bufs=2)
                ssq = spool.tile([128, 1], f32, tag="ssq", bufs=4)
                nc.scalar.activation(sq[:], hs, AF.Square, accum_out=ssq[:])
                mu2n = spool.tile([128, 1], f32, tag="mu2n", bufs=4)
                nc.vector.tensor_tensor(out=mu2n[:], in0=mu[:], in1=mu[:], op=ALU.mult)
                negb = spool.tile([128, 1], f32, tag="negb", bufs=4)
                nc.scalar.activation(negb[:], mu2n[:], AF.Identity, scale=-1.0, bias=c_eps[:])
                stdv = spool.tile([128, 1], f32, tag="stdv", bufs=4)
                nc.scalar.activation(stdv[:], ssq[:], AF.Sqrt, scale=1.0 / H, bias=negb[:])
                rstd = spool.tile([128, 1], f32, tag="rstd", bufs=4)
                nc.vector.reciprocal(rstd[:], stdv[:])
                xn = lnpool.tile([128, H], bf16, tag="xn", bufs=3)
                nc.vector.tensor_scalar(out=xn[:], in0=hs, scalar1=mu[:],
                                        scalar2=rstd[:], op0=ALU.subtract, op1=ALU.mult)
                return xn

            def ln_trans(p, xn, hnfm_t, aff):
                """Transpose xn chunk p to feature-major, applying the affine."""
                for m in range(4):
                    ptr = ppS.tile([128, 128], bf16, tag="ps", bufs=2)
                    nc.tensor.transpose(ptr[:], xn[:, m * 128:(m + 1) * 128], eyeb[:])
                    nc.scalar.activation(
                        hnfm_t[:, m * NPC + p * 128: m * NPC + (p + 1) * 128],
                        ptr[:], AF.Identity, scale=aff[:, m:m + 1], bias=aff[:, 4 + m:5 + m])

            # LN for layer 0 (later layers fuse their LN into the previous
            # layer's node stage so the PE never waits on the stats chain)
            hnfm = lnpool.tile([128, 4 * NPC], bf16, tag="hnfm")
            xns = [ln_stats(p) for p in range(3)]
            for p in range(3):
                ln_trans(p, xns[p], hnfm, wt["bl"])
            dis_tiles = {0: dis_compute(0), 1: dis_compute(1)}

            # ---- layers ----
            for l in range(L):
                wlE, wlN = wt["wlE"], wt["wlN"]
                bl, nb2r, w1cbt = wt["bl"], wt["nb2r"], wt["w1cbt"]

                # A = hn@W1a, B = hn@W1b (fm, fp32); Cb = lat@W1c + e_b1 (fm, per graph)
                AtC = lnpool.tile([128, 4 * NPC], f32, tag="AtC")
                Bt = lnpool.tile([128, 4 * NPC], f32, tag="Bt")
                for which, dst in ((0, AtC), (1, Bt)):
                    for m in range(4):
                        pa = ppZ.tile([128, NPC], f32, tag="pz", bufs=2)
                        for k in range(4):
                            nc.tensor.matmul(
                                pa[:],
                                wlE[:, OFF_AB + (which * 4 + k) * 512 + m * 128:
                                       OFF_AB + (which * 4 + k) * 512 + (m + 1) * 128],
                                hnfm[:, k * NPC:(k + 1) * NPC],
                                start=(k == 0), stop=(k == 3))
                        nc.scalar.copy(dst[:, m * NPC:(m + 1) * NPC], pa[:])
                Cb = spool.tile([128, 4 * GPC], f32, tag="Cb")
                for m in range(4):
                    pc = ppZ.tile([128, GPC], f32, tag="pz", bufs=2)
                    nc.tensor.matmul(pc[:], w1cbt[:, m * 128:(m + 1) * 128], latfm10[:],
                                     start=True, stop=True)
                    nc.scalar.copy(Cb[:, m * GPC:(m + 1) * GPC], pc[:])

                # fold C[g]+e_b1 into the A-broadcast so the te-silu bias is 0
                nc.vector.tensor_tensor(
                    out=AtC[:].rearrange("p (m g a) -> p m g a", g=GPC, a=ATOMS),
                    in0=AtC[:].rearrange("p (m g a) -> p m g a", g=GPC, a=ATOMS),
                    in1=Cb[:].rearrange("p (m g) -> p m g", g=GPC)
                        .rearrange("p m (g o) -> p m g o", o=1)
                        .broadcast_to([128, 4, GPC, ATOMS]),
                    op=ALU.add)

                # agg accumulator (fm, bf16)
                aggb = lnpool.tile([128, 4 * NPC], bf16, tag="aggb")
                aggv = aggb[:].rearrange("p (m n) -> p m n", n=NPC)
                Atv = AtC[:].rearrange("p (m n) -> p m n", n=NPC)
                Btv = Bt[:].rearrange("p (m n) -> p m n", n=NPC)

                def z1_stage(g, hf, dis):
                    """z1 = D + A[src] + B[dst] (+C[g]+b1 folded in A); te = silu(z1)."""
                    n0 = g * ATOMS + hf * NBLK
                    te = epool.tile([128, 4 * HALF], bf16, tag="te", bufs=3)
                    for mp in range(2):
                        m0 = 2 * mp
                        pz = ppZ.tile([128, 1024], f32, tag="pz", bufs=2)
                        for m in (m0, m0 + 1):
                            for k in range(6):
                                nc.tensor.matmul(
                                    pz[:, (m - m0) * 512:(m - m0) * 512 + HALF],
                                    wlE[:, OFF_D + k * 512 + m * 128:
                                          OFF_D + k * 512 + (m + 1) * 128],
                                    dis[:, k * HALF:(k + 1) * HALF],
                                    start=(k == 0), stop=(k == 5))
                        pzv = pz[:].rearrange("p (m q) -> p m q", q=512)[:, :, 0:HALF] \
                                   .rearrange("p m (i j) -> p m i j", j=ATOMS)
                        nc.vector.tensor_tensor(
                            out=pzv, in0=pzv,
                            in1=Atv[:, m0:m0 + 2, n0:n0 + NBLK]
                                .rearrange("p m (i o) -> p m i o", o=1)
                                .broadcast_to([128, 2, NBLK, ATOMS]), op=ALU.add)
                        nc.vector.tensor_tensor(
                            out=pzv, in0=pzv,
                            in1=Btv[:, m0:m0 + 2, g * ATOMS:(g + 1) * ATOMS]
                                .rearrange("p m (o j) -> p m o j", o=1)
                                .broadcast_to([128, 2, NBLK, ATOMS]), op=ALU.add)
                        silu(epool,
                             te[:].rearrange("p (m e) -> p m e", e=HALF)[:, m0:m0 + 2, :],
                             pz[:].rearrange("p (m q) -> p m q", q=512)[:, :, 0:HALF],
                             bias=0.0)
                    return te

                def w2_stage(g, hf, te):
                    """ef = silu(te.T @ e_w2 + e_b2); agg += blockwise sum.

                    The 24-wide block sum runs as a gpsimd add-tree (24->12->6->3
                    ->1+2) to keep the DVE free for the z1 psum adds; only the
                    last tiny add is on the DVE (bf16 output).
                    """
                    n0 = g * ATOMS + hf * NBLK
                    for mp in range(2):
                        m0 = 2 * mp
                        ps = ppS.tile([128, 1024], f32, tag="ps", bufs=2)
                        for m in (m0, m0 + 1):
                            for k in range(4):
                                nc.tensor.matmul(
                                    ps[:, (m - m0) * 512:(m - m0) * 512 + HALF],
                                    wlE[:, OFF_W2 + k * 512 + m * 128:
                                          OFF_W2 + k * 512 + (m + 1) * 128],
                                    te[:, k * HALF:(k + 1) * HALF],
                                    start=(k == 0), stop=(k == 3))
                        ef = epool.tile([128, 2 * HALF], f32, tag="ef", bufs=3)
                        efv = ef[:].rearrange("p (m i j) -> p m i j", j=ATOMS, i=NBLK)
                        silu(epool,
                             ef[:].rearrange("p (m e) -> p m e", e=HALF),
                             ps[:].rearrange("p (m q) -> p m q", q=512)[:, :, 0:HALF],
                             bias=bl[:, 8 + m0:9 + m0],
                             bias2=bl[:, 9 + m0:10 + m0])
                        if (g * 2 + hf) % 2 == 0:
                            # even tiles: plain DVE reduce; odd tiles: gpsimd
                            # add-tree — splits the reduce load so neither
                            # engine outpaces the PE's per-tile budget
                            with nc.allow_low_precision(reason="agg rounds to bf16"):
                                nc.vector.tensor_reduce(
                                    out=aggv[:, m0:m0 + 2, n0:n0 + NBLK],
                                    in_=efv, axis=AX.X, op=ALU.add)
                            continue
                        s1 = epool.tile([128, 2 * NBLK * 12], f32, tag="rs1", bufs=2)
                        s1v = s1[:].rearrange("p (m i j) -> p m i j", i=NBLK, j=12)
                        nc.gpsimd.tensor_tensor(out=s1v, in0=efv[:, :, :, 0:12],
                                                in1=efv[:, :, :, 12:24], op=ALU.add)
                        s2 = epool.tile([128, 2 * NBLK * 6], f32, tag="rs2", bufs=2)
                        s2v = s2[:].rearrange("p (m i j) -> p m i j", i=NBLK, j=6)
                        nc.gpsimd.tensor_tensor(out=s2v, in0=s1v[:, :, :, 0:6],
                                                in1=s1v[:, :, :, 6:12], op=ALU.add)
                        s3 = epool.tile([128, 2 * NBLK * 3], f32, tag="rs3", bufs=2)
                        s3v = s3[:].rearrange("p (m i j) -> p m i j", i=NBLK, j=3)
                        nc.gpsimd.tensor_tensor(out=s3v, in0=s2v[:, :, :, 0:3],
                                                in1=s2v[:, :, :, 3:6], op=ALU.add)
                        s4 = epool.tile([128, 2 * NBLK], f32, tag="rs4", bufs=2)
                        s4v = s4[:].rearrange("p (m i o) -> p m i o", i=NBLK, o=1)
                        nc.gpsimd.tensor_tensor(out=s4v, in0=s3v[:, :, :, 0:1],
                                                in1=s3v[:, :, :, 1:2], op=ALU.add)
                        nc.vector.tensor_tensor(
                            out=aggv[:, m0:m0 + 2, n0:n0 + NBLK]
                                .rearrange("p m (n o) -> p m n o", o=1),
                            in0=s4v, in1=s3v[:, :, :, 2:3], op=ALU.add)

                # ---- edge tiles, software-pipelined (z1 of t before W2 of t-1) ----
                pend = None
                for t in range(NTILE):
                    g, hf = t // 2, t % 2
                    if l == 0:
                        if t + 2 < NTILE:
                            dis_tiles[t + 2] = dis_compute(t + 2)
                        dis = dis_tiles.pop(t)
                    else:
                        dis = epool.tile([128, 6 * HALF], bf16, tag="dis", bufs=4)
                        nc.sync.dma_start(
                            dis[:], dis_d[:, t * 6 * HALF:(t + 1) * 6 * HALF])
                    if t == 2 and l + 1 < L:
                        wt_next = load_weights(l + 1)
                    te = z1_stage(g, hf, dis)
                    if pend is not None:
                        w2_stage(*pend)
                    pend = (g, hf, te)
                w2_stage(*pend)

                # ---- node MLP ----
                t2s = lnpool.tile([128, 4 * NPC], bf16, tag="t2s")
                for m in range(4):
                    pn = ppS.tile([128, NPC], f32, tag="ps", bufs=2)
                    for k in range(8):
                        src_t = hnfm if k < 4 else aggb
                        nc.tensor.matmul(
                            pn[:],
                            wlN[:, OFF_N1 + k * 512 + m * 128:
                                   OFF_N1 + k * 512 + (m + 1) * 128],
                            src_t[:, (k % 4) * NPC:((k % 4) + 1) * NPC],
                            start=(k == 0), stop=(k == 7))
                    silu(lnpool, t2s[:, m * NPC:(m + 1) * NPC], pn[:],
                         bias=bl[:, 12 + m:13 + m])
                # node update + fused LN of the NEXT layer (or the final LN):
                # stats run on ACT/DVE while the PE continues with nw2 matmuls;
                # transposes are batched afterwards so they find xn ready.
                nxt_aff = wt_next["bl"] if l + 1 < L else flnt
                hnfm_next = lnpool.tile([128, 4 * NPC], bf16, tag="hnfm")
                xns = []
                for p in range(3):
                    pd = ppS.tile([128, H], f32, tag="ps", bufs=2)
                    for k in range(4):
                        nc.tensor.matmul(
                            pd[:],
                            t2s[:, k * NPC + p * 128: k * NPC + (p + 1) * 128],
                            wlN[:, OFF_N2 + k * 512:OFF_N2 + (k + 1) * 512],
                            start=(k == 0), stop=False)
                    nc.tensor.matmul(pd[:], ones1[:], nb2r[:], start=False, stop=True)
                    ds = lnpool.tile([128, H], f32, tag="ds", bufs=2)
                    silu(lnpool, ds[:], pd[:])
                    nc.vector.tensor_tensor(out=hbuf[:, p * H:(p + 1) * H],
                                            in0=hbuf[:, p * H:(p + 1) * H],
                                            in1=ds[:], op=ALU.add)
                    xns.append(ln_stats(p))
                for p in range(3):
                    ln_trans(p, xns[p], hnfm_next, nxt_aff)
                hnfm = hnfm_next
                if l + 1 < L:
                    wt = wt_next

            # ---- head (hnfm now holds the final-LN output) ----
            hfn = hnfm
            for p in range(3):
                pd = ppS.tile([128, H], f32, tag="ps", bufs=2)
                for k in range(4):
                    nc.tensor.matmul(
                        pd[:],
                        hfn[:, k * NPC + p * 128: k * NPC + (p + 1) * 128],
                        outw[:, k * H:(k + 1) * H],
                        start=(k == 0), stop=False)
                nc.tensor.matmul(pd[:], ones1[:], outbr[:], start=False, stop=True)
                ho = lnpool.tile([128, H], f32, tag="ho", bufs=2)
                nc.scalar.copy(ho[:], pd[:])
                nc.sync.dma_start(hout[p * 128:(p + 1) * 128, :], ho[:])

    return nc


def make_in_maps(inputs):
    """Full inputs -> per-core input dicts (graph-parallel sharding)."""
    import ml_dtypes
    BF = ml_dtypes.bfloat16
    at = np.asarray(inputs["atom_types"])
    fc = np.asarray(inputs["frac_coords"], np.float32)
    lat = np.asarray(inputs["lattices"], np.float32)
    nw1 = np.array(inputs["n_w1"], np.float32)
    nw1[:, H:, :] *= 1.0 / ATOMS  # fold scatter_mean denominator
    ew1 = np.asarray(inputs["e_w1"], np.float32)
    w1cb = np.concatenate(
        [ew1[:, 1024:1033, :],
         np.asarray(inputs["e_b1"], np.float32)[:, None, :]], axis=1)
    shared = {
        "eye": np.eye(128, dtype=np.float32),
        "eyeb": np.eye(128, dtype=np.float32).astype(BF),
        "ones1": np.ones((1, 128), BF),
        "fcol": np.arange(128, dtype=np.float32).reshape(128, 1),
        "embt": np.asarray(inputs["emb_table"], np.float32),
        "ew1": ew1.astype(BF),
        "ew2": np.asarray(inputs["e_w2"], np.float32).astype(BF),
        "w1cb": w1cb.astype(BF),
        "nw1": nw1.astype(BF),
        "nw2": np.asarray(inputs["n_w2"], np.float32).astype(BF),
        "lnw": np.asarray(inputs["ln_w"], np.float32),
        "lnb": np.asarray(inputs["ln_b"], np.float32),
        "eb2": np.asarray(inputs["e_b2"], np.float32),
        "nb1": np.asarray(inputs["n_b1"], np.float32),
        "nb2": np.asarray(inputs["n_b2"], np.float32).astype(BF),
        "flnw": np.asarray(inputs["fln_w"], np.float32),
        "flnb": np.asarray(inputs["fln_b"], np.float32),
        "outw": np.asarray(inputs["out_w"], np.float32).astype(BF),
        "outb": np.asarray(inputs["out_b"], np.float32).astype(BF),
    }
    in_maps = []
    for c in range(NCORES):
        nsl = slice(c * NPC, (c + 1) * NPC)
        oneh = np.zeros((MAXEL, NPC), np.float32)
        oneh[at[nsl], np.arange(NPC)] = 1.0
        m = dict(shared)
        m["xT"] = np.ascontiguousarray(fc[nsl].T)
        m["oneh"] = oneh
        m["lat"] = np.ascontiguousarray(lat[c * GPC:(c + 1) * GPC].reshape(GPC, 9))
        in_maps.append(m)
    return in_maps


_CACHE = {}


def kernel(**inputs) -> np.ndarray:
    from concourse.bass_utils import run_bass_kernel_spmd

    if "nc" not in _CACHE:
        nc = bacc.Bacc("TRN2", debug=False)
        build(nc, sim_silu=False)
        nc.compile()
        _CACHE["nc"] = nc
    res = run_bass_kernel_spmd(_CACHE["nc"], make_in_maps(inputs),
                               core_ids=list(range(NCORES)))
    return np.concatenate([np.asarray(r["hout"]) for r in res.results], axis=0)


# revision 17
# speedup vs baseline: 1.0044x; 1.0044x over previous
"""Trainium2 Bass kernel for CSPNet-style GNN message passing (128 graphs x 24 atoms).

Strategy (graph-parallel over 8 cores, 16 graphs/core):
  - Edges are fully-connected per graph (24x24 incl. self loops) -> deg=24, and the
    edge MLP's first matmul decomposes over e_in = [hn[src], hn[dst], lat_e, dis]:
        z1 = A[src] + B[dst] + C[g] + dis @ W1d + b1
    with A = hn@W1a, B = hn@W1b computed at NODE level (24x fewer flops), and the
    src/dst gathers realized as zero-stride broadcast access patterns (no data mvmt).
  - dis (sin/cos positional features) computed once into DRAM as bf16; range-reduced
    via (f*t mod 1) so ACT Sin sees [-pi, pi].
  - All big matmuls run in bf16 (full PE rate + fast weight load); psum fp32.
  - Feature-major ("fm") layout [feat_on_partitions, tokens] for all matmul chains;
    node-major only for LayerNorm; PE-transpose bridges the two once per layer.
  - scatter_mean folds into a free-dim strided reduce (blocks of 24) + 1/24 folded
    into n_w1 rows on the host.
  - Edge loop software-pipelined: z1 of tile t is issued before W2 of tile t-1 so
    the PE never waits on the DVE-add + silu chain of the current tile.
  - Per-layer weights double-buffered and prefetched from inside the previous
    layer's edge loop.
"""

import os
import sys

import numpy as np

if "/opt/trn_rl_repo" not in sys.path:
    sys.path.insert(0, "/opt/trn_rl_repo")

import concourse.bass as bass
import concourse.tile as tile
from concourse import bacc, mybir

f32 = mybir.dt.float32
bf16 = mybir.dt.bfloat16
i32 = mybir.dt.int32
AF = mybir.ActivationFunctionType
ALU = mybir.AluOpType
AX = mybir.AxisListType

N_GRAPHS = 128
ATOMS = 24
N = N_GRAPHS * ATOMS
H = 512
L = 6
NFREQ = 128
MAXEL = 100
NCORES = 8
GPC = N_GRAPHS // NCORES          # 16 graphs per core
NPC = GPC * ATOMS                 # 384 nodes per core
EPC = GPC * ATOMS * ATOMS         # 9216 edges per core
HALF = ATOMS * ATOMS // 2         # 288 edges per tile (12 src blocks)
NBLK = 12                         # src blocks per half-graph tile
NTILE = 2 * GPC                   # 32 edge tiles per layer

# column offsets inside the edge weight tile wlE [128, 9216] bf16
OFF_AB = 0              # 8 x 512   (W1a k=0..3, W1b k=0..3)
OFF_D = 8 * 512         # 6 x 512   (W1d)
OFF_W2 = OFF_D + 6 * 512   # 4 x 512
WLE_COLS = OFF_W2 + 4 * 512   # 9216
# node weight tile wlN [128, 6144] bf16
OFF_N1 = 0              # 8 x 512
OFF_N2 = 8 * 512        # 4 x 512
WLN_COLS = OFF_N2 + 4 * 512   # 6144


def build(nc: bass.Bass, sim_silu: bool = False):
    """Trace the per-core program. Same program for all 8 cores (SPMD)."""
    din = {}
    for name, shape, dt in [
        ("xT", [3, NPC], f32), ("oneh", [MAXEL, NPC], f32), ("lat", [GPC, 9], f32),
        ("eye", [128, 128], f32), ("eyeb", [128, 128], bf16), ("fcol", [128, 1], f32),
        ("embt", [MAXEL, H], f32),
        ("ew1", [L, 2 * H + 9 + 6 * NFREQ, H], bf16), ("ew2", [L, H, H], bf16),
        ("w1cb", [L, 10, H], bf16),
        ("nw1", [L, 2 * H, H], bf16), ("nw2", [L, H, H], bf16),
        ("lnw", [L, H], f32), ("lnb", [L, H], f32),
        ("eb2", [L, H], f32), ("nb1", [L, H], f32), ("nb2", [L, H], bf16),
        ("flnw", [H], f32), ("flnb", [H], f32),
        ("outw", [H, H], bf16), ("outb", [H], bf16),
        ("ones1", [1, 128], bf16),
    ]:
        din[name] = nc.dram_tensor(name, shape, dt, kind="ExternalInput")
    hout = nc.dram_tensor("hout", [NPC, H], f32, kind="ExternalOutput")

    def silu1(pool, out_ap, in_ap, bias=0.0):
        """out = Silu(in + bias). bias: [128,1] AP or float."""
        if not sim_silu:
            nc.scalar.activation(out_ap, in_ap, AF.Silu, bias=bias, scale=1.0)
        else:  # CoreSim lacks Silu: Identity(+bias) -> Sigmoid -> mul
            t1 = pool.tile(list(in_ap.shape), f32, tag="sims1", bufs=2)
            nc.scalar.activation(t1[:], in_ap, AF.Identity, bias=bias, scale=1.0)
            t2 = pool.tile(list(in_ap.shape), f32, tag="sims2", bufs=2)
            nc.scalar.activation(t2[:], t1[:], AF.Sigmoid)
            nc.vector.tensor_tensor(out=out_ap, in0=t1[:], in1=t2[:], op=ALU.mult)

    def silu(pool, out_ap, in_ap, bias=0.0, bias2=None):
        """Silu over [128, 2, E] m-pair views when bias2 given, else single."""
        if bias2 is None:
            silu1(pool, out_ap, in_ap, bias)
        else:
            silu1(pool, out_ap[:, 0], in_ap[:, 0], bias)
            silu1(pool, out_ap[:, 1], in_ap[:, 1], bias2)

    with tile.TileContext(nc) as tc:
        with (
            tc.tile_pool(name="const", bufs=1) as cpool,
            tc.tile_pool(name="wl", bufs=1) as wpool,
            tc.tile_pool(name="node", bufs=1) as npool,
            tc.tile_pool(name="ln", bufs=1) as lnpool,
            tc.tile_pool(name="edge", bufs=1) as epool,
            tc.tile_pool(name="small", bufs=1) as spool,
            tc.tile_pool(name="dram", bufs=1, space="DRAM") as dpool,
            tc.tile_pool(name="psZ", bufs=1, space="PSUM") as ppZ,
            tc.tile_pool(name="psS", bufs=1, space="PSUM") as ppS,
        ):
            # ---- constants / setup ----
            eye = cpool.tile([128, 128], f32)
            nc.sync.dma_start(eye[:], din["eye"][:])
            eyeb = cpool.tile([128, 128], bf16)
            nc.sync.dma_start(eyeb[:], din["eyeb"][:])
            fcol = cpool.tile([128, 1], f32)
            nc.sync.dma_start(fcol[:], din["fcol"][:])
            ones1 = cpool.tile([1, 128], bf16)
            nc.sync.dma_start(ones1[:], din["ones1"][:])
            c_eps = cpool.tile([128, 1], f32)
            nc.vector.memset(c_eps[:], 1e-5)
            c_hpi = cpool.tile([128, 1], f32)
            nc.vector.memset(c_hpi[:], float(np.pi / 2))
            c_pi = cpool.tile([128, 1], f32)
            nc.vector.memset(c_pi[:], float(np.pi))
            xt = cpool.tile([3, NPC], f32)
            nc.sync.dma_start(xt[:], din["xT"][:])
            oneh = cpool.tile([MAXEL, NPC], f32)
            nc.sync.dma_start(oneh[:], din["oneh"][:])
            embt = cpool.tile([MAXEL, H], f32)
            nc.sync.dma_start(embt[:], din["embt"][:])
            latt = cpool.tile([GPC, 9], f32)
            nc.sync.dma_start(latt[:], din["lat"][:])
            outw = cpool.tile([128, 4 * H], bf16)
            for k in range(4):
                nc.sync.dma_start(outw[:, k * H:(k + 1) * H],
                                  din["outw"][k * 128:(k + 1) * 128, :])
            outbr = cpool.tile([1, H], bf16)
            nc.sync.dma_start(outbr[:], din["outb"][:].rearrange("(o h) -> o h", o=1))
            flnt = cpool.tile([128, 8], f32)
            nc.sync.dma_start(flnt[:, 0:4], din["flnw"][:].rearrange("(m p) -> p m", p=128))
            nc.sync.dma_start(flnt[:, 4:8], din["flnb"][:].rearrange("(m p) -> p m", p=128))

            # fd[c, e=(g,i,j)] = x[dst=j, c] - x[src=i, c] + 1  (stored in DRAM)
            fd_d = dpool.tile([3, EPC], f32)
            for g in range(GPC):
                sl = xt[:, g * ATOMS:(g + 1) * ATOMS]
                fds = spool.tile([3, ATOMS * ATOMS], f32, tag="fds", bufs=1)
                nc.vector.tensor_tensor(
                    out=fds[:].rearrange("p (i j) -> p i j", j=ATOMS),
                    in0=sl.rearrange("p (o j) -> p o j", o=1).broadcast_to([3, ATOMS, ATOMS]),
                    in1=sl.broadcast_to([3, ATOMS, ATOMS]),
                    op=ALU.subtract)
                nc.vector.tensor_scalar(out=fds[:], in0=fds[:], scalar1=1.0,
                                        scalar2=None, op0=ALU.add)
                nc.sync.dma_start(fd_d[:, g * ATOMS * ATOMS:(g + 1) * ATOMS * ATOMS], fds[:])

            # dis features (layer-independent), computed per tile, stored bf16:
            # r = f*t - rne(f*t) in [-0.5,0.5];  sin chunk = Sin(2*pi*r);
            # cos chunk = Sin(pi/2 - 2*pi*|r|) = cos(2*pi*r).  HW Sin is only
            # accurate for |arg| <= pi (measured), so the abs trick is required.
            # Work is spread over ACT (mult via scale, abs, sins), DVE (int
            # round-trip) and GPSIMD (subtract) so layer 0 can interleave this
            # with its edge tiles without starving any one engine.
            dis_d = dpool.tile([128, NTILE * 6 * HALF], bf16)

            def dis_compute(t):
                """Emit setup for dis tile t; returns the SBUF bf16 tile."""
                e0 = t * HALF
                emb3 = epool.tile([128, 3 * HALF], f32, tag="emb3", bufs=2)
                nc.sync.dma_start(
                    emb3[:].rearrange("p (c e) -> p c e", e=HALF),
                    fd_d[:, e0:e0 + HALF].partition_broadcast(128))
                embm = epool.tile([128, 3 * HALF], f32, tag="embm", bufs=2)
                nc.scalar.activation(embm[:], emb3[:], AF.Identity, scale=fcol[:])
                vi = epool.tile([128, 3 * HALF], i32, tag="vi", bufs=2)
                nc.vector.tensor_copy(vi[:], embm[:])
                nc.gpsimd.tensor_tensor(out=embm[:], in0=embm[:], in1=vi[:],
                                        op=ALU.subtract)
                rabs = epool.tile([128, 3 * HALF], f32, tag="rabs", bufs=2)
                nc.scalar.activation(rabs[:], embm[:], AF.Abs)
                diss = epool.tile([128, 6 * HALF], bf16, tag="diss", bufs=3)
                for c in range(3):
                    nc.scalar.activation(diss[:, c * HALF:(c + 1) * HALF],
                                         embm[:, c * HALF:(c + 1) * HALF],
                                         AF.Sin, scale=2 * np.pi)
                    nc.scalar.activation(diss[:, (3 + c) * HALF:(4 + c) * HALF],
                                         rabs[:, c * HALF:(c + 1) * HALF],
                                         AF.Sin, scale=-2 * np.pi, bias=c_hpi[:])
                nc.sync.dma_start(dis_d[:, t * 6 * HALF:(t + 1) * 6 * HALF], diss[:])
                return diss

            # lattice gram: lat_ip[g, i*3+k] = sum_j L[g,i,j] L[g,k,j]
            # -> latfm10 [10, GPC] bf16 with a trailing ones row (for the +e_b1 fold)
            lt27 = spool.tile([GPC, 27], f32, tag="lt27")
            nc.vector.tensor_tensor(
                out=lt27[:].rearrange("p (i k j) -> p i k j", k=3, j=3),
                in0=latt[:].rearrange("p (i o j) -> p i o j", o=1, j=3)
                    .broadcast_to([GPC, 3, 3, 3]),
                in1=latt[:].rearrange("p (o k j) -> p o k j", o=1, j=3)
                    .broadcast_to([GPC, 3, 3, 3]),
                op=ALU.mult)
            latnm = spool.tile([GPC, 9], f32, tag="latnm")
            nc.vector.tensor_reduce(out=latnm[:], in_=lt27[:].rearrange(
                "p (ik j) -> p ik j", j=3), axis=AX.X, op=ALU.add)
            plat = ppS.tile([9, GPC], f32, tag="ps", bufs=2)
            nc.tensor.transpose(plat[:], latnm[:], eye[:GPC, :GPC])
            latfm10 = cpool.tile([10, GPC], bf16)
            nc.vector.memset(latfm10[:], 1.0)
            nc.scalar.copy(latfm10[0:9, :], plat[:])

            # h0 = emb_table[atom_types]  (node-major, via one-hot matmul, exact fp32)
            hbuf = npool.tile([128, 3 * H], f32)
            for p in range(3):
                ph = ppS.tile([128, H], f32, tag="ps", bufs=2)
                nc.tensor.matmul(ph[:], oneh[:, p * 128:(p + 1) * 128], embt[:],
                                 start=True, stop=True)
                nc.scalar.copy(hbuf[:, p * H:(p + 1) * H], ph[:])

            def load_weights(l):
                """DMA layer l weights into double-buffered SBUF tiles."""
                wlE = wpool.tile([128, WLE_COLS], bf16, tag="wlE", bufs=2)
                wlN = wpool.tile([128, WLN_COLS], bf16, tag="wlN", bufs=2)

                def wload(dst, off, dram_rows, nk):
                    nc.sync.dma_start(
                        dst[:, off:off + nk * 512].rearrange("p (k m) -> p k m", m=512),
                        dram_rows.rearrange("(k p) m -> p k m", p=128))
                wload(wlE, OFF_AB, din["ew1"][l, 0:1024, :], 8)        # W1a+W1b
                wload(wlE, OFF_D, din["ew1"][l, 1033:1801, :], 6)      # W1d
                wload(wlE, OFF_W2, din["ew2"][l], 4)                   # e_w2
                wload(wlN, OFF_N1, din["nw1"][l], 8)                   # n_w1 (pre-scaled)
                wload(wlN, OFF_N2, din["nw2"][l], 4)                   # n_w2
                w1cbt = wpool.tile([10, H], bf16, tag="w1cb", bufs=2)
                nc.sync.dma_start(w1cbt[:], din["w1cb"][l])
                bl = wpool.tile([128, 16], f32, tag="bl", bufs=2)
                for j, nm in enumerate(["lnw", "lnb", "eb2", "nb1"]):
                    nc.sync.dma_start(bl[:, j * 4:(j + 1) * 4],
                                      din[nm][l].rearrange("(m p) -> p m", p=128))
                nb2r = wpool.tile([1, H], bf16, tag="nb2r", bufs=2)
                nc.sync.dma_start(nb2r[:], din["nb2"][l].rearrange("(o h) -> o h", o=1))
                return dict(wlE=wlE, wlN=wlN, w1cbt=w1cbt, bl=bl, nb2r=nb2r)

            wt = load_weights(0)

            def ln_stats(p):
                """LayerNorm stats for node chunk p of hbuf -> xn (bf16)."""
                hs = hbuf[:, p * H:(p + 1) * H]
                musum = spool.tile([128, 1], f32, tag="musum", bufs=4)
                nc.vector.tensor_reduce(out=musum[:], in_=hs, axis=AX.X, op=ALU.add)
                mu = spool.tile([128, 1], f32, tag="mu", bufs=4)
                nc.scalar.activation(mu[:], musum[:], AF.Identity, scale=1.0 / H)
                sq = lnpool.tile([128, H], f32, tag="sq", bufs=2)
                ssq = spool.tile([128, 1], f32, tag="ssq", bufs=4)
                nc.scalar.activation(sq[:], hs, AF.Square, accum_out=ssq[:])
                mu2n = spool.tile([128, 1], f32, tag="mu2n", bufs=4)
                nc.vector.tensor_tensor(out=mu2n[:], in0=mu[:], in1=mu[:], op=ALU.mult)
                negb = spool.tile([128, 1], f32, tag="negb", bufs=4)
                nc.scalar.activation(negb[:], mu2n[:], AF.Identity, scale=-1.0, bias=c_eps[:])
                stdv = spool.tile([128, 1], f32, tag="stdv", bufs=4)
                nc.scalar.activation(stdv[:], ssq[:], AF.Sqrt, scale=1.0 / H, bias=negb[:])
                rstd = spool.tile([128, 1], f32, tag="rstd", bufs=4)
                nc.vector.reciprocal(rstd[:], stdv[:])
                xn = lnpool.tile([128, H], bf16, tag="xn", bufs=3)
                nc.vector.tensor_scalar(out=xn[:], in0=hs, scalar1=mu[:],
                                        scalar2=rstd[:], op0=ALU.subtract, op1=ALU.mult)
                return xn

            def ln_trans(p, xn, hnfm_t, aff):
                """Transpose xn chunk p to feature-major, applying the affine."""
                for m in range(4):
                    ptr = ppS.tile([128, 128], bf16, tag="ps", bufs=2)
                    nc.tensor.transpose(ptr[:], xn[:, m * 128:(m + 1) * 128], eyeb[:])
                    nc.scalar.activation(
                        hnfm_t[:, m * NPC + p * 128: m * NPC + (p + 1) * 128],
                        ptr[:], AF.Identity, scale=aff[:, m:m + 1], bias=aff[:, 4 + m:5 + m])

            # LN for layer 0 (later layers fuse their LN into the previous
            # layer's node stage so the PE never waits on the stats chain)
            hnfm = lnpool.tile([128, 4 * NPC], bf16, tag="hnfm")
            xns = [ln_stats(p) for p in range(3)]
            for p in range(3):
                ln_trans(p, xns[p], hnfm, wt["bl"])
            dis_tiles = {0: dis_compute(0), 1: dis_compute(1)}

            # ---- layers ----
            for l in range(L):
                wlE, wlN = wt["wlE"], wt["wlN"]
                bl, nb2r, w1cbt = wt["bl"], wt["nb2r"], wt["w1cbt"]

                # A = hn@W1a, B = hn@W1b (fm, fp32); Cb = lat@W1c + e_b1 (fm, per graph)
                AtC = lnpool.tile([128, 4 * NPC], f32, tag="AtC")
                Bt = lnpool.tile([128, 4 * NPC], f32, tag="Bt")
                for which, dst in ((0, AtC), (1, Bt)):
                    for m in range(4):
                        pa = ppZ.tile([128, NPC], f32, tag="pz", bufs=2)
                        for k in range(4):
                            nc.tensor.matmul(
                                pa[:],
                                wlE[:, OFF_AB + (which * 4 + k) * 512 + m * 128:
                                       OFF_AB + (which * 4 + k) * 512 + (m + 1) * 128],
                                hnfm[:, k * NPC:(k + 1) * NPC],
                                start=(k == 0), stop=(k == 3))
                        nc.scalar.copy(dst[:, m * NPC:(m + 1) * NPC], pa[:])
                Cb = spool.tile([128, 4 * GPC], f32, tag="Cb")
                for m in range(4):
                    pc = ppZ.tile([128, GPC], f32, tag="pz", bufs=2)
                    nc.tensor.matmul(pc[:], w1cbt[:, m * 128:(m + 1) * 128], latfm10[:],
                                     start=True, stop=True)
                    nc.scalar.copy(Cb[:, m * GPC:(m + 1) * GPC], pc[:])

                # fold C[g]+e_b1 into the A-broadcast so the te-silu bias is 0
                nc.vector.tensor_tensor(
                    out=AtC[:].rearrange("p (m g a) -> p m g a", g=GPC, a=ATOMS),
                    in0=AtC[:].rearrange("p (m g a) -> p m g a", g=GPC, a=ATOMS),
                    in1=Cb[:].rearrange("p (m g) -> p m g", g=GPC)
                        .rearrange("p m (g o) -> p m g o", o=1)
                        .broadcast_to([128, 4, GPC, ATOMS]),
                    op=ALU.add)

                # agg accumulator (fm, bf16)
                aggb = lnpool.tile([128, 4 * NPC], bf16, tag="aggb")
                aggv = aggb[:].rearrange("p (m n) -> p m n", n=NPC)
                Atv = AtC[:].rearrange("p (m n) -> p m n", n=NPC)
                Btv = Bt[:].rearrange("p (m n) -> p m n", n=NPC)

                def z1_stage(g, hf, dis):
                    """z1 = D + A[src] + B[dst] (+C[g]+b1 folded in A); te = silu(z1)."""
                    n0 = g * ATOMS + hf * NBLK
                    te = epool.tile([128, 4 * HALF], bf16, tag="te", bufs=3)
                    for mp in range(2):
                        m0 = 2 * mp
                        pz = ppZ.tile([128, 1024], f32, tag="pz", bufs=2)
                        for m in (m0, m0 + 1):
                            for k in range(6):
                                nc.tensor.matmul(
                                    pz[:, (m - m0) * 512:(m - m0) * 512 + HALF],
                                    wlE[:, OFF_D + k * 512 + m * 128:
                                          OFF_D + k * 512 + (m + 1) * 128],
                                    dis[:, k * HALF:(k + 1) * HALF],
                                    start=(k == 0), stop=(k == 5))
                        pzv = pz[:].rearrange("p (m q) -> p m q", q=512)[:, :, 0:HALF] \
                                   .rearrange("p m (i j) -> p m i j", j=ATOMS)
                        nc.vector.tensor_tensor(
                            out=pzv, in0=pzv,
                            in1=Atv[:, m0:m0 + 2, n0:n0 + NBLK]
                                .rearrange("p m (i o) -> p m i o", o=1)
                                .broadcast_to([128, 2, NBLK, ATOMS]), op=ALU.add)
                        nc.vector.tensor_tensor(
                            out=pzv, in0=pzv,
                            in1=Btv[:, m0:m0 + 2, g * ATOMS:(g + 1) * ATOMS]
                                .rearrange("p m (o j) -> p m o j", o=1)
                                .broadcast_to([128, 2, NBLK, ATOMS]), op=ALU.add)
                        silu(epool,
                             te[:].rearrange("p (m e) -> p m e", e=HALF)[:, m0:m0 + 2, :],
                             pz[:].rearrange("p (m q) -> p m q", q=512)[:, :, 0:HALF],
                             bias=0.0)
                    return te

                def w2_stage(g, hf, te):
                    """ef = silu(te.T @ e_w2 + e_b2); agg += blockwise sum.

                    The 24-wide block sum runs as a gpsimd add-tree (24->12->6->3
                    ->1+2) to keep the DVE free for the z1 psum adds; only the
                    last tiny add is on the DVE (bf16 output).
                    """
                    n0 = g * ATOMS + hf * NBLK
                    for mp in range(2):
                        m0 = 2 * mp
                        ps = ppS.tile([128, 1024], f32, tag="ps", bufs=2)
                        for m in (m0, m0 + 1):
                            for k in range(4):
                                nc.tensor.matmul(
                                    ps[:, (m - m0) * 512:(m - m0) * 512 + HALF],
                                    wlE[:, OFF_W2 + k * 512 + m * 128:
                                          OFF_W2 + k * 512 + (m + 1) * 128],
                                    te[:, k * HALF:(k + 1) * HALF],
                                    start=(k == 0), stop=(k == 3))
                        ef = epool.tile([128, 2 * HALF], f32, tag="ef", bufs=3)
                        efv = ef[:].rearrange("p (m i j) -> p m i j", j=ATOMS, i=NBLK)
                        silu(epool,
                             ef[:].rearrange("p (m e) -> p m e", e=HALF),
                             ps[:].rearrange("p (m q) -> p m q", q=512)[:, :, 0:HALF],
                             bias=bl[:, 8 + m0:9 + m0],
                             bias2=bl[:, 9 + m0:10 + m0])
                        if (g * 2 + hf) % 2 == 0:
                            # even tiles: plain DVE reduce; odd tiles: gpsimd
                            # add-tree — splits the reduce load so neither
                            # engine outpaces the PE's per-tile budget
                            with nc.allow_low_precision(reason="agg rounds to bf16"):
                                nc.vector.tensor_reduce(
                                    out=aggv[:, m0:m0 + 2, n0:n0 + NBLK],
                                    in_=efv, axis=AX.X, op=ALU.add)
                            continue
                        s1 = epool.tile([128, 2 * NBLK * 12], f32, tag="rs1", bufs=2)
                        s1v = s1[:].rearrange("p (m i j) -> p m i j", i=NBLK, j=12)
                        nc.gpsimd.tensor_tensor(out=s1v, in0=efv[:, :, :, 0:12],
                                                in1=efv[:, :, :, 12:24], op=ALU.add)
                        s2 = epool.tile([128, 2 * NBLK * 6], f32, tag="rs2", bufs=2)
                        s2v = s2[:].rearrange("p (m i j) -> p m i j", i=NBLK, j=6)
                        nc.gpsimd.tensor_tensor(out=s2v, in0=s1v[:, :, :, 0:6],
                                                in1=s1v[:, :, :, 6:12], op=ALU.add)
                        s3 = epool.tile([128, 2 * NBLK * 3], f32, tag="rs3", bufs=2)
                        s3v = s3[:].rearrange("p (m i j) -> p m i j", i=NBLK, j=3)
                        nc.gpsimd.tensor_tensor(out=s3v, in0=s2v[:, :, :, 0:3],
                                                in1=s2v[:, :, :, 3:6], op=ALU.add)
                        s4 = epool.tile([128, 2 * NBLK], f32, tag="rs4", bufs=2)
                        s4v = s4[:].rearrange("p (m i o) -> p m i o", i=NBLK, o=1)
                        nc.gpsimd.tensor_tensor(out=s4v, in0=s3v[:, :, :, 0:1],
                                                in1=s3v[:, :, :, 1:2], op=ALU.add)
                        nc.vector.tensor_tensor(
                            out=aggv[:, m0:m0 + 2, n0:n0 + NBLK]
                                .rearrange("p m (n o) -> p m n o", o=1),
                            in0=s4v, in1=s3v[:, :, :, 2:3], op=ALU.add)

                # ---- edge tiles, software-pipelined two deep: z1 of tile t is
                # issued before W2 of tile t-2, so the te chain (psum adds +
                # silu, ~3.6us) has two full tile slots of slack before the PE
                # needs it back ----
                pend = []
                for t in range(NTILE):
                    g, hf = t // 2, t % 2
                    if l == 0:
                        if t + 2 < NTILE:
                            dis_tiles[t + 2] = dis_compute(t + 2)
                        dis = dis_tiles.pop(t)
                    else:
                        dis = epool.tile([128, 6 * HALF], bf16, tag="dis", bufs=4)
                        nc.sync.dma_start(
                            dis[:], dis_d[:, t * 6 * HALF:(t + 1) * 6 * HALF])
                    if t == 2 and l + 1 < L:
                        wt_next = load_weights(l + 1)
                    te = z1_stage(g, hf, dis)
                    pend.append((g, hf, te))
                    if len(pend) > 2:
                        w2_stage(*pend.pop(0))
                for args in pend:
                    w2_stage(*args)

                # ---- node MLP ----
                t2s = lnpool.tile([128, 4 * NPC], bf16, tag="t2s")
                for m in range(4):
                    pn = ppS.tile([128, NPC], f32, tag="ps", bufs=2)
                    for k in range(8):
                        src_t = hnfm if k < 4 else aggb
                        nc.tensor.matmul(
                            pn[:],
                            wlN[:, OFF_N1 + k * 512 + m * 128:
                                   OFF_N1 + k * 512 + (m + 1) * 128],
                            src_t[:, (k % 4) * NPC:((k % 4) + 1) * NPC],
                            start=(k == 0), stop=(k == 7))
                    silu(lnpool, t2s[:, m * NPC:(m + 1) * NPC], pn[:],
                         bias=bl[:, 12 + m:13 + m])
                # node update + fused LN of the NEXT layer (or the final LN):
                # stats run on ACT/DVE while the PE continues with nw2 matmuls;
                # transposes are batched afterwards so they find xn ready.
                nxt_aff = wt_next["bl"] if l + 1 < L else flnt
                hnfm_next = lnpool.tile([128, 4 * NPC], bf16, tag="hnfm")
                xns = []
                for p in range(3):
                    pd = ppS.tile([128, H], f32, tag="ps", bufs=2)
                    for k in range(4):
                        nc.tensor.matmul(
                            pd[:],
                            t2s[:, k * NPC + p * 128: k * NPC + (p + 1) * 128],
                            wlN[:, OFF_N2 + k * 512:OFF_N2 + (k + 1) * 512],
                            start=(k == 0), stop=False)
                    nc.tensor.matmul(pd[:], ones1[:], nb2r[:], start=False, stop=True)
                    ds = lnpool.tile([128, H], f32, tag="ds", bufs=2)
                    silu(lnpool, ds[:], pd[:])
                    nc.vector.tensor_tensor(out=hbuf[:, p * H:(p + 1) * H],
                                            in0=hbuf[:, p * H:(p + 1) * H],
                                            in1=ds[:], op=ALU.add)
                    xns.append(ln_stats(p))
                for p in range(3):
                    ln_trans(p, xns[p], hnfm_next, nxt_aff)
                hnfm = hnfm_next
                if l + 1 < L:
                    wt = wt_next

            # ---- head (hnfm now holds the final-LN output) ----
            hfn = hnfm
            for p in range(3):
                pd = ppS.tile([128, H], f32, tag="ps", bufs=2)
                for k in range(4):
                    nc.tensor.matmul(
                        pd[:],
                        hfn[:, k * NPC + p * 128: k * NPC + (p + 1) * 128],
                        outw[:, k * H:(k + 1) * H],
                        start=(k == 0), stop=False)
                nc.tensor.matmul(pd[:], ones1[:], outbr[:], start=False, stop=True)
                ho = lnpool.tile([128, H], f32, tag="ho", bufs=2)
                nc.scalar.copy(ho[:], pd[:])
                nc.sync.dma_start(hout[p * 128:(p + 1) * 128, :], ho[:])

    return nc


def make_in_maps(inputs):
    """Full inputs -> per-core input dicts (graph-parallel sharding)."""
    import ml_dtypes
    BF = ml_dtypes.bfloat16
    at = np.asarray(inputs["atom_types"])
    fc = np.asarray(inputs["frac_coords"], np.float32)
    lat = np.asarray(inputs["lattices"], np.float32)
    nw1 = np.array(inputs["n_w1"], np.float32)
    nw1[:, H:, :] *= 1.0 / ATOMS  # fold scatter_mean denominator
    ew1 = np.asarray(inputs["e_w1"], np.float32)
    w1cb = np.concatenate(
        [ew1[:, 1024:1033, :],
         np.asarray(inputs["e_b1"], np.float32)[:, None, :]], axis=1)
    shared = {
        "eye": np.eye(128, dtype=np.float32),
        "eyeb": np.eye(128, dtype=np.float32).astype(BF),
        "ones1": np.ones((1, 128), BF),
        "fcol": np.arange(128, dtype=np.float32).reshape(128, 1),
        "embt": np.asarray(inputs["emb_table"], np.float32),
        "ew1": ew1.astype(BF),
        "ew2": np.asarray(inputs["e_w2"], np.float32).astype(BF),
        "w1cb": w1cb.astype(BF),
        "nw1": nw1.astype(BF),
        "nw2": np.asarray(inputs["n_w2"], np.float32).astype(BF),
        "lnw": np.asarray(inputs["ln_w"], np.float32),
        "lnb": np.asarray(inputs["ln_b"], np.float32),
        "eb2": np.asarray(inputs["e_b2"], np.float32),
        "nb1": np.asarray(inputs["n_b1"], np.float32),
        "nb2": np.asarray(inputs["n_b2"], np.float32).astype(BF),
        "flnw": np.asarray(inputs["fln_w"], np.float32),
        "flnb": np.asarray(inputs["fln_b"], np.float32),
        "outw": np.asarray(inputs["out_w"], np.float32).astype(BF),
        "outb": np.asarray(inputs["out_b"], np.float32).astype(BF),
    }
    in_maps = []
    for c in range(NCORES):
        nsl = slice(c * NPC, (c + 1) * NPC)
        oneh = np.zeros((MAXEL, NPC), np.float32)
        oneh[at[nsl], np.arange(NPC)] = 1.0
        m = dict(shared)
        m["xT"] = np.ascontiguousarray(fc[nsl].T)
        m["oneh"] = oneh
        m["lat"] = np.ascontiguousarray(lat[c * GPC:(c + 1) * GPC].reshape(GPC, 9))
        in_maps.append(m)
    return in_maps


_CACHE = {}


def kernel(**inputs) -> np.ndarray:
    from concourse.bass_utils import run_bass_kernel_spmd

    if "nc" not in _CACHE:
        nc = bacc.Bacc("TRN2", debug=False)
        build(nc, sim_silu=False)
        nc.compile()
        _CACHE["nc"] = nc
    res = run_bass_kernel_spmd(_CACHE["nc"], make_in_maps(inputs),
                               core_ids=list(range(NCORES)))
    return np.concatenate([np.asarray(r["hout"]) for r in res.results], axis=0)


# revision 19
# speedup vs baseline: 1.0798x; 1.0750x over previous
"""Trainium2 Bass kernel for CSPNet-style GNN message passing (128 graphs x 24 atoms).

Strategy (graph-parallel over 8 cores, 16 graphs/core):
  - Edges are fully-connected per graph (24x24 incl. self loops) -> deg=24, and the
    edge MLP's first matmul decomposes over e_in = [hn[src], hn[dst], lat_e, dis]:
        z1 = A[src] + B[dst] + C[g] + dis @ W1d + b1
    with A = hn@W1a, B = hn@W1b computed at NODE level (24x fewer flops), and the
    src/dst gathers realized as zero-stride broadcast access patterns (no data mvmt).
  - dis (sin/cos positional features) computed once into DRAM as bf16; range-reduced
    via (f*t mod 1) so ACT Sin sees [-pi, pi].
  - All big matmuls run in bf16 (full PE rate + fast weight load); psum fp32.
  - Feature-major ("fm") layout [feat_on_partitions, tokens] for all matmul chains;
    node-major only for LayerNorm; PE-transpose bridges the two once per layer.
  - scatter_mean folds into a free-dim strided reduce (blocks of 24) + 1/24 folded
    into n_w1 rows on the host.
  - Edge loop software-pipelined: z1 of tile t is issued before W2 of tile t-1 so
    the PE never waits on the DVE-add + silu chain of the current tile.
  - Per-layer weights double-buffered and prefetched from inside the previous
    layer's edge loop.
"""

import os
import sys

import numpy as np

if "/opt/trn_rl_repo" not in sys.path:
    sys.path.insert(0, "/opt/trn_rl_repo")

import concourse.bass as bass
import concourse.tile as tile
from concourse import bacc, mybir

f32 = mybir.dt.float32
bf16 = mybir.dt.bfloat16
i32 = mybir.dt.int32
AF = mybir.ActivationFunctionType
ALU = mybir.AluOpType
AX = mybir.AxisListType

N_GRAPHS = 128
ATOMS = 24
N = N_GRAPHS * ATOMS
H = 512
L = 6
NFREQ = 128
MAXEL = 100
NCORES = 8
GPC = N_GRAPHS // NCORES          # 16 graphs per core
NPC = GPC * ATOMS                 # 384 nodes per core
EPC = GPC * ATOMS * ATOMS         # 9216 edges per core
HALF = ATOMS * ATOMS // 2         # 288 edges per tile (12 src blocks)
NBLK = 12                         # src blocks per half-graph tile
NTILE = 2 * GPC                   # 32 edge tiles per layer

# column offsets inside the edge weight tile wlE [128, 9216] bf16
OFF_AB = 0              # 8 x 512   (W1a k=0..3, W1b k=0..3)
OFF_D = 8 * 512         # 6 x 512   (W1d)
OFF_W2 = OFF_D + 6 * 512   # 4 x 512
WLE_COLS = OFF_W2 + 4 * 512   # 9216
# node weight tile wlN [128, 6144] bf16
OFF_N1 = 0              # 8 x 512
OFF_N2 = 8 * 512        # 4 x 512
WLN_COLS = OFF_N2 + 4 * 512   # 6144


def build(nc: bass.Bass, sim_silu: bool = False):
    """Trace the per-core program. Same program for all 8 cores (SPMD)."""
    din = {}
    for name, shape, dt in [
        ("xT", [3, NPC], f32), ("oneh", [MAXEL, NPC], f32), ("lat", [GPC, 9], f32),
        ("eye", [128, 128], f32), ("eyeb", [128, 128], bf16), ("fcol", [128, 1], f32),
        ("embt", [MAXEL, H], f32),
        ("ew1", [L, 2 * H + 9 + 6 * NFREQ, H], bf16), ("ew2", [L, H, H], bf16),
        ("w1cb", [L, 10, H], bf16),
        ("nw1", [L, 2 * H, H], bf16), ("nw2", [L, H, H], bf16),
        ("lnw", [L, H], f32), ("lnb", [L, H], f32),
        ("eb2", [L, H], f32), ("nb1", [L, H], f32), ("nb2", [L, H], bf16),
        ("flnw", [H], f32), ("flnb", [H], f32),
        ("outw", [H, H], bf16), ("outb", [H], bf16),
        ("ones1", [1, 128], bf16),
    ]:
        din[name] = nc.dram_tensor(name, shape, dt, kind="ExternalInput")
    hout = nc.dram_tensor("hout", [NPC, H], f32, kind="ExternalOutput")

    def silu1(pool, out_ap, in_ap, bias=0.0):
        """out = Silu(in + bias). bias: [128,1] AP or float."""
        if not sim_silu:
            nc.scalar.activation(out_ap, in_ap, AF.Silu, bias=bias, scale=1.0)
        else:  # CoreSim lacks Silu: Identity(+bias) -> Sigmoid -> mul
            t1 = pool.tile(list(in_ap.shape), f32, tag="sims1", bufs=2)
            nc.scalar.activation(t1[:], in_ap, AF.Identity, bias=bias, scale=1.0)
            t2 = pool.tile(list(in_ap.shape), f32, tag="sims2", bufs=2)
            nc.scalar.activation(t2[:], t1[:], AF.Sigmoid)
            nc.vector.tensor_tensor(out=out_ap, in0=t1[:], in1=t2[:], op=ALU.mult)

    def silu(pool, out_ap, in_ap, bias=0.0, bias2=None):
        """Silu over [128, 2, E] m-pair views when bias2 given, else single."""
        if bias2 is None:
            silu1(pool, out_ap, in_ap, bias)
        else:
            silu1(pool, out_ap[:, 0], in_ap[:, 0], bias)
            silu1(pool, out_ap[:, 1], in_ap[:, 1], bias2)

    with tile.TileContext(nc) as tc:
        with (
            tc.tile_pool(name="const", bufs=1) as cpool,
            tc.tile_pool(name="wl", bufs=1) as wpool,
            tc.tile_pool(name="node", bufs=1) as npool,
            tc.tile_pool(name="ln", bufs=1) as lnpool,
            tc.tile_pool(name="edge", bufs=1) as epool,
            tc.tile_pool(name="small", bufs=1) as spool,
            tc.tile_pool(name="dram", bufs=1, space="DRAM") as dpool,
            tc.tile_pool(name="psZ", bufs=1, space="PSUM") as ppZ,
            tc.tile_pool(name="psS", bufs=1, space="PSUM") as ppS,
        ):
            # ---- constants / setup ----
            eye = cpool.tile([128, 128], f32)
            nc.sync.dma_start(eye[:], din["eye"][:])
            eyeb = cpool.tile([128, 128], bf16)
            nc.sync.dma_start(eyeb[:], din["eyeb"][:])
            fcol = cpool.tile([128, 1], f32)
            nc.sync.dma_start(fcol[:], din["fcol"][:])
            ones1 = cpool.tile([1, 128], bf16)
            nc.sync.dma_start(ones1[:], din["ones1"][:])
            c_eps = cpool.tile([128, 1], f32)
            nc.vector.memset(c_eps[:], 1e-5)
            c_hpi = cpool.tile([128, 1], f32)
            nc.vector.memset(c_hpi[:], float(np.pi / 2))
            c_pi = cpool.tile([128, 1], f32)
            nc.vector.memset(c_pi[:], float(np.pi))
            xt = cpool.tile([3, NPC], f32)
            nc.sync.dma_start(xt[:], din["xT"][:])
            oneh = cpool.tile([MAXEL, NPC], f32)
            nc.sync.dma_start(oneh[:], din["oneh"][:])
            embt = cpool.tile([MAXEL, H], f32)
            nc.sync.dma_start(embt[:], din["embt"][:])
            latt = cpool.tile([GPC, 9], f32)
            nc.sync.dma_start(latt[:], din["lat"][:])
            outw = cpool.tile([128, 4 * H], bf16)
            for k in range(4):
                nc.sync.dma_start(outw[:, k * H:(k + 1) * H],
                                  din["outw"][k * 128:(k + 1) * 128, :])
            outbr = cpool.tile([1, H], bf16)
            nc.sync.dma_start(outbr[:], din["outb"][:].rearrange("(o h) -> o h", o=1))
            flnt = cpool.tile([128, 8], f32)
            nc.sync.dma_start(flnt[:, 0:4], din["flnw"][:].rearrange("(m p) -> p m", p=128))
            nc.sync.dma_start(flnt[:, 4:8], din["flnb"][:].rearrange("(m p) -> p m", p=128))

            # fd[c, e=(g,i,j)] = x[dst=j, c] - x[src=i, c] + 1  (stored in DRAM)
            fd_d = dpool.tile([3, EPC], f32)
            for g in range(GPC):
                sl = xt[:, g * ATOMS:(g + 1) * ATOMS]
                fds = spool.tile([3, ATOMS * ATOMS], f32, tag="fds", bufs=1)
                nc.vector.tensor_tensor(
                    out=fds[:].rearrange("p (i j) -> p i j", j=ATOMS),
                    in0=sl.rearrange("p (o j) -> p o j", o=1).broadcast_to([3, ATOMS, ATOMS]),
                    in1=sl.broadcast_to([3, ATOMS, ATOMS]),
                    op=ALU.subtract)
                nc.vector.tensor_scalar(out=fds[:], in0=fds[:], scalar1=1.0,
                                        scalar2=None, op0=ALU.add)
                nc.sync.dma_start(fd_d[:, g * ATOMS * ATOMS:(g + 1) * ATOMS * ATOMS], fds[:])

            # dis features (layer-independent), computed per tile, stored bf16:
            # r = f*t - rne(f*t) in [-0.5,0.5];  sin chunk = Sin(2*pi*r);
            # cos chunk = Sin(pi/2 - 2*pi*|r|) = cos(2*pi*r).  HW Sin is only
            # accurate for |arg| <= pi (measured), so the abs trick is required.
            # Work is spread over ACT (mult via scale, abs, sins), DVE (int
            # round-trip) and GPSIMD (subtract) so layer 0 can interleave this
            # with its edge tiles without starving any one engine.
            dis_d = dpool.tile([128, NTILE * 6 * HALF], bf16)

            def dis_compute(t):
                """Emit setup for dis tile t; returns the SBUF bf16 tile."""
                e0 = t * HALF
                emb3 = epool.tile([128, 3 * HALF], f32, tag="emb3", bufs=2)
                nc.sync.dma_start(
                    emb3[:].rearrange("p (c e) -> p c e", e=HALF),
                    fd_d[:, e0:e0 + HALF].partition_broadcast(128))
                nc.vector.tensor_scalar(out=emb3[:], in0=emb3[:], scalar1=fcol[:],
                                        scalar2=None, op0=ALU.mult)
                vi = epool.tile([128, 3 * HALF], i32, tag="vi", bufs=2)
                nc.gpsimd.tensor_copy(vi[:], emb3[:])
                nc.gpsimd.tensor_tensor(out=emb3[:], in0=emb3[:], in1=vi[:],
                                        op=ALU.subtract)
                rabs = epool.tile([128, 3 * HALF], f32, tag="rabs", bufs=2)
                nc.scalar.activation(rabs[:], emb3[:], AF.Abs)
                diss = epool.tile([128, 6 * HALF], bf16, tag="diss", bufs=3)
                for c in range(3):
                    nc.scalar.activation(diss[:, c * HALF:(c + 1) * HALF],
                                         emb3[:, c * HALF:(c + 1) * HALF],
                                         AF.Sin, scale=2 * np.pi)
                    nc.scalar.activation(diss[:, (3 + c) * HALF:(4 + c) * HALF],
                                         rabs[:, c * HALF:(c + 1) * HALF],
                                         AF.Sin, scale=-2 * np.pi, bias=c_hpi[:])
                nc.sync.dma_start(dis_d[:, t * 6 * HALF:(t + 1) * 6 * HALF], diss[:])
                return diss

            # lattice gram: lat_ip[g, i*3+k] = sum_j L[g,i,j] L[g,k,j]
            # -> latfm10 [10, GPC] bf16 with a trailing ones row (for the +e_b1 fold)
            lt27 = spool.tile([GPC, 27], f32, tag="lt27")
            nc.vector.tensor_tensor(
                out=lt27[:].rearrange("p (i k j) -> p i k j", k=3, j=3),
                in0=latt[:].rearrange("p (i o j) -> p i o j", o=1, j=3)
                    .broadcast_to([GPC, 3, 3, 3]),
                in1=latt[:].rearrange("p (o k j) -> p o k j", o=1, j=3)
                    .broadcast_to([GPC, 3, 3, 3]),
                op=ALU.mult)
            latnm = spool.tile([GPC, 9], f32, tag="latnm")
            nc.vector.tensor_reduce(out=latnm[:], in_=lt27[:].rearrange(
                "p (ik j) -> p ik j", j=3), axis=AX.X, op=ALU.add)
            plat = ppS.tile([9, GPC], f32, tag="ps", bufs=2)
            nc.tensor.transpose(plat[:], latnm[:], eye[:GPC, :GPC])
            latfm10 = cpool.tile([10, GPC], bf16)
            nc.vector.memset(latfm10[:], 1.0)
            nc.scalar.copy(latfm10[0:9, :], plat[:])

            # h0 = emb_table[atom_types]  (node-major, via one-hot matmul, exact fp32)
            hbuf = npool.tile([128, 3 * H], f32)
            for p in range(3):
                ph = ppS.tile([128, H], f32, tag="ps", bufs=2)
                nc.tensor.matmul(ph[:], oneh[:, p * 128:(p + 1) * 128], embt[:],
                                 start=True, stop=True)
                nc.scalar.copy(hbuf[:, p * H:(p + 1) * H], ph[:])

            def load_weights(l):
                """DMA layer l weights into double-buffered SBUF tiles."""
                wlE = wpool.tile([128, WLE_COLS], bf16, tag="wlE", bufs=2)
                wlN = wpool.tile([128, WLN_COLS], bf16, tag="wlN", bufs=2)

                def wload(dst, off, dram_rows, nk):
                    nc.sync.dma_start(
                        dst[:, off:off + nk * 512].rearrange("p (k m) -> p k m", m=512),
                        dram_rows.rearrange("(k p) m -> p k m", p=128))
                wload(wlE, OFF_AB, din["ew1"][l, 0:1024, :], 8)        # W1a+W1b
                wload(wlE, OFF_D, din["ew1"][l, 1033:1801, :], 6)      # W1d
                wload(wlE, OFF_W2, din["ew2"][l], 4)                   # e_w2
                wload(wlN, OFF_N1, din["nw1"][l], 8)                   # n_w1 (pre-scaled)
                wload(wlN, OFF_N2, din["nw2"][l], 4)                   # n_w2
                w1cbt = wpool.tile([10, H], bf16, tag="w1cb", bufs=2)
                nc.sync.dma_start(w1cbt[:], din["w1cb"][l])
                bl = wpool.tile([128, 16], f32, tag="bl", bufs=2)
                for j, nm in enumerate(["lnw", "lnb", "eb2", "nb1"]):
                    nc.sync.dma_start(bl[:, j * 4:(j + 1) * 4],
                                      din[nm][l].rearrange("(m p) -> p m", p=128))
                nb2r = wpool.tile([1, H], bf16, tag="nb2r", bufs=2)
                nc.sync.dma_start(nb2r[:], din["nb2"][l].rearrange("(o h) -> o h", o=1))
                return dict(wlE=wlE, wlN=wlN, w1cbt=w1cbt, bl=bl, nb2r=nb2r)

            wt = load_weights(0)

            def ln_stats(p):
                """LayerNorm stats for node chunk p of hbuf -> xn (bf16)."""
                hs = hbuf[:, p * H:(p + 1) * H]
                musum = spool.tile([128, 1], f32, tag="musum", bufs=4)
                nc.vector.tensor_reduce(out=musum[:], in_=hs, axis=AX.X, op=ALU.add)
                mu = spool.tile([128, 1], f32, tag="mu", bufs=4)
                nc.scalar.activation(mu[:], musum[:], AF.Identity, scale=1.0 / H)
                sq = lnpool.tile([128, H], f32, tag="sq", bufs=2)
                ssq = spool.tile([128, 1], f32, tag="ssq", bufs=4)
                nc.scalar.activation(sq[:], hs, AF.Square, accum_out=ssq[:])
                mu2n = spool.tile([128, 1], f32, tag="mu2n", bufs=4)
                nc.vector.tensor_tensor(out=mu2n[:], in0=mu[:], in1=mu[:], op=ALU.mult)
                negb = spool.tile([128, 1], f32, tag="negb", bufs=4)
                nc.scalar.activation(negb[:], mu2n[:], AF.Identity, scale=-1.0, bias=c_eps[:])
                stdv = spool.tile([128, 1], f32, tag="stdv", bufs=4)
                nc.scalar.activation(stdv[:], ssq[:], AF.Sqrt, scale=1.0 / H, bias=negb[:])
                rstd = spool.tile([128, 1], f32, tag="rstd", bufs=4)
                nc.vector.reciprocal(rstd[:], stdv[:])
                xn = lnpool.tile([128, H], bf16, tag="xn", bufs=3)
                nc.vector.tensor_scalar(out=xn[:], in0=hs, scalar1=mu[:],
                                        scalar2=rstd[:], op0=ALU.subtract, op1=ALU.mult)
                return xn

            def ln_trans(p, xn, hnfm_t, aff):
                """Transpose xn chunk p to feature-major, applying the affine."""
                for m in range(4):
                    ptr = ppS.tile([128, 128], bf16, tag="ps", bufs=2)
                    nc.tensor.transpose(ptr[:], xn[:, m * 128:(m + 1) * 128], eyeb[:])
                    nc.scalar.activation(
                        hnfm_t[:, m * NPC + p * 128: m * NPC + (p + 1) * 128],
                        ptr[:], AF.Identity, scale=aff[:, m:m + 1], bias=aff[:, 4 + m:5 + m])

            # LN for layer 0 (later layers fuse their LN into the previous
            # layer's node stage so the PE never waits on the stats chain)
            hnfm = lnpool.tile([128, 4 * NPC], bf16, tag="hnfm")
            xns = [ln_stats(p) for p in range(3)]
            for p in range(3):
                ln_trans(p, xns[p], hnfm, wt["bl"])
            dis_tiles = {0: dis_compute(0), 1: dis_compute(1)}

            # ---- layers ----
            for l in range(L):
                wlE, wlN = wt["wlE"], wt["wlN"]
                bl, nb2r, w1cbt = wt["bl"], wt["nb2r"], wt["w1cbt"]

                # A = hn@W1a, B = hn@W1b (fm, fp32); Cb = lat@W1c + e_b1 (fm, per graph)
                AtC = lnpool.tile([128, 4 * NPC], f32, tag="AtC")
                Bt = lnpool.tile([128, 4 * NPC], f32, tag="Bt")
                for which, dst in ((0, AtC), (1, Bt)):
                    for m in range(4):
                        pa = ppZ.tile([128, NPC], f32, tag="pz", bufs=2)
                        for k in range(4):
                            nc.tensor.matmul(
                                pa[:],
                                wlE[:, OFF_AB + (which * 4 + k) * 512 + m * 128:
                                       OFF_AB + (which * 4 + k) * 512 + (m + 1) * 128],
                                hnfm[:, k * NPC:(k + 1) * NPC],
                                start=(k == 0), stop=(k == 3))
                        nc.scalar.copy(dst[:, m * NPC:(m + 1) * NPC], pa[:])
                Cb = spool.tile([128, 4 * GPC], f32, tag="Cb")
                for m in range(4):
                    pc = ppZ.tile([128, GPC], f32, tag="pz", bufs=2)
                    nc.tensor.matmul(pc[:], w1cbt[:, m * 128:(m + 1) * 128], latfm10[:],
                                     start=True, stop=True)
                    nc.scalar.copy(Cb[:, m * GPC:(m + 1) * GPC], pc[:])

                # fold C[g]+e_b1 into the A-broadcast so the te-silu bias is 0
                nc.vector.tensor_tensor(
                    out=AtC[:].rearrange("p (m g a) -> p m g a", g=GPC, a=ATOMS),
                    in0=AtC[:].rearrange("p (m g a) -> p m g a", g=GPC, a=ATOMS),
                    in1=Cb[:].rearrange("p (m g) -> p m g", g=GPC)
                        .rearrange("p m (g o) -> p m g o", o=1)
                        .broadcast_to([128, 4, GPC, ATOMS]),
                    op=ALU.add)

                # agg accumulator (fm, bf16)
                aggb = lnpool.tile([128, 4 * NPC], bf16, tag="aggb")
                aggv = aggb[:].rearrange("p (m n) -> p m n", n=NPC)
                Atv = AtC[:].rearrange("p (m n) -> p m n", n=NPC)
                Btv = Bt[:].rearrange("p (m n) -> p m n", n=NPC)

                def z1_stage(g, hf, dis):
                    """z1 = D + A[src] + B[dst] (+C[g]+b1 folded in A); te = silu(z1)."""
                    n0 = g * ATOMS + hf * NBLK
                    te = epool.tile([128, 4 * HALF], bf16, tag="te", bufs=3)
                    for mp in range(2):
                        m0 = 2 * mp
                        pz = ppZ.tile([128, 1024], f32, tag="pz", bufs=2)
                        for m in (m0, m0 + 1):
                            for k in range(6):
                                nc.tensor.matmul(
                                    pz[:, (m - m0) * 512:(m - m0) * 512 + HALF],
                                    wlE[:, OFF_D + k * 512 + m * 128:
                                          OFF_D + k * 512 + (m + 1) * 128],
                                    dis[:, k * HALF:(k + 1) * HALF],
                                    start=(k == 0), stop=(k == 5))
                        pzv = pz[:].rearrange("p (m q) -> p m q", q=512)[:, :, 0:HALF] \
                                   .rearrange("p m (i j) -> p m i j", j=ATOMS)
                        nc.vector.tensor_tensor(
                            out=pzv, in0=pzv,
                            in1=Atv[:, m0:m0 + 2, n0:n0 + NBLK]
                                .rearrange("p m (i o) -> p m i o", o=1)
                                .broadcast_to([128, 2, NBLK, ATOMS]), op=ALU.add)
                        nc.vector.tensor_tensor(
                            out=pzv, in0=pzv,
                            in1=Btv[:, m0:m0 + 2, g * ATOMS:(g + 1) * ATOMS]
                                .rearrange("p m (o j) -> p m o j", o=1)
                                .broadcast_to([128, 2, NBLK, ATOMS]), op=ALU.add)
                        silu(epool,
                             te[:].rearrange("p (m e) -> p m e", e=HALF)[:, m0:m0 + 2, :],
                             pz[:].rearrange("p (m q) -> p m q", q=512)[:, :, 0:HALF],
                             bias=0.0)
                    return te

                def w2_stage(g, hf, te):
                    """ef = silu(te.T @ e_w2 + e_b2); agg += blockwise sum.

                    The 24-wide block sum runs as a gpsimd add-tree (24->12->6->3
                    ->1+2) to keep the DVE free for the z1 psum adds; only the
                    last tiny add is on the DVE (bf16 output).
                    """
                    n0 = g * ATOMS + hf * NBLK
                    for mp in range(2):
                        m0 = 2 * mp
                        ps = ppS.tile([128, 1024], f32, tag="ps", bufs=2)
                        for m in (m0, m0 + 1):
                            for k in range(4):
                                nc.tensor.matmul(
                                    ps[:, (m - m0) * 512:(m - m0) * 512 + HALF],
                                    wlE[:, OFF_W2 + k * 512 + m * 128:
                                          OFF_W2 + k * 512 + (m + 1) * 128],
                                    te[:, k * HALF:(k + 1) * HALF],
                                    start=(k == 0), stop=(k == 3))
                        ef = epool.tile([128, 2 * HALF], bf16, tag="ef", bufs=4)
                        silu(epool,
                             ef[:].rearrange("p (m e) -> p m e", e=HALF),
                             ps[:].rearrange("p (m q) -> p m q", q=512)[:, :, 0:HALF],
                             bias=bl[:, 8 + m0:9 + m0],
                             bias2=bl[:, 9 + m0:10 + m0])
                        with nc.allow_low_precision(reason="agg rounds to bf16"):
                            nc.vector.tensor_reduce(
                                out=aggv[:, m0:m0 + 2, n0:n0 + NBLK],
                                in_=ef[:].rearrange("p (m i j) -> p m i j", j=ATOMS, i=NBLK),
                                axis=AX.X, op=ALU.add)

                # ---- edge tiles, software-pipelined (z1 of t before W2 of t-1) ----
                pend = []
                for t in range(NTILE):
                    g, hf = t // 2, t % 2
                    if l == 0:
                        if t + 2 < NTILE:
                            dis_tiles[t + 2] = dis_compute(t + 2)
                        dis = dis_tiles.pop(t)
                    else:
                        dis = epool.tile([128, 6 * HALF], bf16, tag="dis", bufs=4)
                        nc.sync.dma_start(
                            dis[:], dis_d[:, t * 6 * HALF:(t + 1) * 6 * HALF])
                    if t == 2 and l + 1 < L:
                        wt_next = load_weights(l + 1)
                    te = z1_stage(g, hf, dis)
                    pend.append((g, hf, te))
                    if len(pend) > 1:
                        w2_stage(*pend.pop(0))
                for args in pend:
                    w2_stage(*args)

                # ---- node MLP ----
                t2s = lnpool.tile([128, 4 * NPC], bf16, tag="t2s")
                for m in range(4):
                    pn = ppS.tile([128, NPC], f32, tag="ps", bufs=2)
                    for k in range(8):
                        src_t = hnfm if k < 4 else aggb
                        nc.tensor.matmul(
                            pn[:],
                            wlN[:, OFF_N1 + k * 512 + m * 128:
                                   OFF_N1 + k * 512 + (m + 1) * 128],
                            src_t[:, (k % 4) * NPC:((k % 4) + 1) * NPC],
                            start=(k == 0), stop=(k == 7))
                    silu(lnpool, t2s[:, m * NPC:(m + 1) * NPC], pn[:],
                         bias=bl[:, 12 + m:13 + m])
                # node update + fused LN of the NEXT layer (or the final LN):
                # stats run on ACT/DVE while the PE continues with nw2 matmuls;
                # transposes are batched afterwards so they find xn ready.
                nxt_aff = wt_next["bl"] if l + 1 < L else flnt
                hnfm_next = lnpool.tile([128, 4 * NPC], bf16, tag="hnfm")
                xns = []
                for p in range(3):
                    pd = ppS.tile([128, H], f32, tag="ps", bufs=2)
                    for k in range(4):
                        nc.tensor.matmul(
                            pd[:],
                            t2s[:, k * NPC + p * 128: k * NPC + (p + 1) * 128],
                            wlN[:, OFF_N2 + k * 512:OFF_N2 + (k + 1) * 512],
                            start=(k == 0), stop=False)
                    nc.tensor.matmul(pd[:], ones1[:], nb2r[:], start=False, stop=True)
                    ds = lnpool.tile([128, H], f32, tag="ds", bufs=2)
                    silu(lnpool, ds[:], pd[:])
                    nc.vector.tensor_tensor(out=hbuf[:, p * H:(p + 1) * H],
                                            in0=hbuf[:, p * H:(p + 1) * H],
                                            in1=ds[:], op=ALU.add)
                    xns.append(ln_stats(p))
                for p in range(3):
                    ln_trans(p, xns[p], hnfm_next, nxt_aff)
                hnfm = hnfm_next
                if l + 1 < L:
                    wt = wt_next

            # ---- head (hnfm now holds the final-LN output) ----
            hfn = hnfm
            for p in range(3):
                pd = ppS.tile([128, H], f32, tag="ps", bufs=2)
                for k in range(4):
                    nc.tensor.matmul(
                        pd[:],
                        hfn[:, k * NPC + p * 128: k * NPC + (p + 1) * 128],
                        outw[:, k * H:(k + 1) * H],
                        start=(k == 0), stop=False)
                nc.tensor.matmul(pd[:], ones1[:], outbr[:], start=False, stop=True)
                ho = lnpool.tile([128, H], f32, tag="ho", bufs=2)
                nc.scalar.copy(ho[:], pd[:])
                nc.sync.dma_start(hout[p * 128:(p + 1) * 128, :], ho[:])

    return nc


def make_in_maps(inputs):
    """Full inputs -> per-core input dicts (graph-parallel sharding)."""
    import ml_dtypes
    BF = ml_dtypes.bfloat16
    at = np.asarray(inputs["atom_types"])
    fc = np.asarray(inputs["frac_coords"], np.float32)
    lat = np.asarray(inputs["lattices"], np.float32)
    nw1 = np.array(inputs["n_w1"], np.float32)
    nw1[:, H:, :] *= 1.0 / ATOMS  # fold scatter_mean denominator
    ew1 = np.asarray(inputs["e_w1"], np.float32)
    w1cb = np.concatenate(
        [ew1[:, 1024:1033, :],
         np.asarray(inputs["e_b1"], np.float32)[:, None, :]], axis=1)
    shared = {
        "eye": np.eye(128, dtype=np.float32),
        "eyeb": np.eye(128, dtype=np.float32).astype(BF),
        "ones1": np.ones((1, 128), BF),
        "fcol": np.arange(128, dtype=np.float32).reshape(128, 1),
        "embt": np.asarray(inputs["emb_table"], np.float32),
        "ew1": ew1.astype(BF),
        "ew2": np.asarray(inputs["e_w2"], np.float32).astype(BF),
        "w1cb": w1cb.astype(BF),
        "nw1": nw1.astype(BF),
        "nw2": np.asarray(inputs["n_w2"], np.float32).astype(BF),
        "lnw": np.asarray(inputs["ln_w"], np.float32),
        "lnb": np.asarray(inputs["ln_b"], np.float32),
        "eb2": np.asarray(inputs["e_b2"], np.float32),
        "nb1": np.asarray(inputs["n_b1"], np.float32),
        "nb2": np.asarray(inputs["n_b2"], np.float32).astype(BF),
        "flnw": np.asarray(inputs["fln_w"], np.float32),
        "flnb": np.asarray(inputs["fln_b"], np.float32),
        "outw": np.asarray(inputs["out_w"], np.float32).astype(BF),
        "outb": np.asarray(inputs["out_b"], np.float32).astype(BF),
    }
    in_maps = []
    for c in range(NCORES):
        nsl = slice(c * NPC, (c + 1) * NPC)
        oneh = np.zeros((MAXEL, NPC), np.float32)
        oneh[at[nsl], np.arange(NPC)] = 1.0
        m = dict(shared)
        m["xT"] = np.ascontiguousarray(fc[nsl].T)
        m["oneh"] = oneh
        m["lat"] = np.ascontiguousarray(lat[c * GPC:(c + 1) * GPC].reshape(GPC, 9))
        in_maps.append(m)
    return in_maps


_CACHE = {}


def kernel(**inputs) -> np.ndarray:
    from concourse.bass_utils import run_bass_kernel_spmd

    if "nc" not in _CACHE:
        nc = bacc.Bacc("TRN2", debug=False)
        build(nc, sim_silu=False)
        nc.compile()
        _CACHE["nc"] = nc
    res = run_bass_kernel_spmd(_CACHE["nc"], make_in_maps(inputs),
                               core_ids=list(range(NCORES)))
    return np.concatenate([np.asarray(r["hout"]) for r in res.results], axis=0)
